# revision 12
# baseline (speedup 1.0000x reference)
"""Bottleneck residual block (1x1 -> 3x3 -> 1x1 conv + BN + residual) on 8 NeuronCores.

Strategy: pure data-parallel over the batch dim (16 images -> 2 per core).
All convs are exact-integer arithmetic in disguise (int8 activations x
small power-of-two int weights), so matmuls are exact in any float format
wide enough: stage 1 runs bf16; stages 2+3 run fp8e4m3 with DoubleRow
(inner activations <= ~14 int, weights in {-4..4} are e4m3-exact).

v2 structure (vs the 60.5us baseline):
  - x is DMA'd as int8 (1.57MB not 3.2MB) and converted i8->bf16 on the
    otherwise-idle DVE/ACT/GPS engines during the head.
  - out is u8 (relu comes free in the fp32->u8 saturating convert).
  - stage 1 runs image-0 first (pass A) then image-1, so the stage-1
    epilogue for image 0 hides entirely under pass B's matmuls.
  - stage-3 image-0 matmuls are interleaved into stage-2 image-1's tap
    stream so their ACT/DVE/GPS epilogues overlap stage-2 PE time.
  - epilogues are spread across ACT+DVE+GPS via static engine tables:
      stage1/2: A = relu(rne(a*psum+b)) as ACT->i16 or DVE->u8(sat),
                cast to fp8 on the other of DVE/GPS.
      stage3:   A = rne(a*psum+b)->i16 (ACT/DVE/GPS tensor_scalar),
                B = r + x (DVE tensor_tensor, i16+bf16, exact),
                C = min(r2,127)->u8 (saturating convert clamps at 0).
  - PSUM: two pools of 2x[128,2,512]: psA holds the long-lived stage-1/2
    accumulators, psB cycles warmup + stage-3 tiles (stage-3 image-1 also
    borrows psA slots once stage 2 is done).

Shapes hardcoded for N=16, Cin=Cout=1024, width=256, H=W=28.
"""

import numpy as np
import ml_dtypes

BF16 = ml_dtypes.bfloat16
FP8 = ml_dtypes.float8_e4m3

N_CORES = 8
N_PER_CORE = 2          # images per core
HW_ = 28 * 28           # 784 spatial positions per image
F = N_PER_CORE * HW_    # 1568 free-dim elements per core
FB = 392                # matmul free-dim block (14 rows of 28)

N_WARMUP = 16           # dummy matmuls before chunk0 lands

# x-chunk i8->bf16 conversion engine per image-half (image-0 halves first,
# they gate stage-1 pass A; image-1 halves later).  GPSIMD ALU/copy ops are
# ~5-30x slower than DVE and poison concurrent DVE throughput - never use
# it for per-element work.
CONV_A = ["dve", "act", "dve", "act", "dve", "act", "dve", "act"]
CONV_B = ["act", "dve", "act", "dve", "act", "dve", "act", "dve"]

# stage-3 epilogue engine tables, iter t = i*8 + m
# (GPSIMD cannot read PSUM; A is ACT/DVE only, B/C are DVE)
A_ENG = ["act"] * 16
B_ENG = ["dve"] * 16
C_ENG = ["dve"] * 16

_CACHE = {}


def _build():
    """Build + compile the per-core Bass kernel once per process."""
    import concourse.bacc as bacc
    import concourse.mybir as mybir
    import concourse.tile as tile

    dt = mybir.dt
    f32, bf16, i16, i8, u8, fp8 = (dt.float32, dt.bfloat16, dt.int16,
                                   dt.int8, dt.uint8, dt.float8e4)
    Alu = mybir.AluOpType
    Act = mybir.ActivationFunctionType
    DR = mybir.MatmulPerfMode.DoubleRowSwInterleave

    nc = bacc.Bacc("TRN2", target_bir_lowering=False, debug=False,
                   num_devices=N_CORES, enable_partition_id=False)

    x_d = nc.dram_tensor("x", [8, 128, F], i8, kind="ExternalInput")
    w1_d = nc.dram_tensor("w1", [128, 16, 128], bf16, kind="ExternalInput")
    w2_d = nc.dram_tensor("w2", [128, 18, 2, 128], fp8, kind="ExternalInput")
    w3_d = nc.dram_tensor("w3", [128, 8, 2, 128], fp8, kind="ExternalInput")
    vec_d = nc.dram_tensor("vec", [128, 24], f32, kind="ExternalInput")
    out_d = nc.dram_tensor("out", [8, 128, F], u8, kind="ExternalOutput")

    with tile.TileContext(nc) as tc:
        with (
            tc.tile_pool(name="persist", bufs=1) as pp,
            tc.tile_pool(name="stage", bufs=4) as sp,
            tc.tile_pool(name="stage3", bufs=8) as sp3,
            tc.tile_pool(name="psA", bufs=2, space="PSUM") as psA,
            tc.tile_pool(name="psB", bufs=2, space="PSUM") as psB,
        ):
            # ---- persistent SBUF tensors ----
            dummy = pp.tile([128, 256], bf16, tag="dummy", name="dummy")
            xi = [pp.tile([128, F], i8, tag=f"xi{k}", name=f"xi{k}")
                  for k in range(8)]
            x_sb = [pp.tile([128, F], bf16, tag=f"x{k}", name=f"x{k}")
                    for k in range(8)]
            w1_sb = pp.tile([128, 16, 128], bf16, tag="w1", name="w1")
            w2_sb = pp.tile([128, 18, 2, 128], fp8, tag="w2", name="w2")
            w3_sb = pp.tile([128, 8, 2, 128], fp8, tag="w3", name="w3")
            vec_sb = pp.tile([128, 24], f32, tag="vec", name="vec")
            s1p = pp.tile([128, 2, 2, 30, 32], fp8, tag="s1p", name="s1p")
            s2f = pp.tile([128, 2, 4, 400], fp8, tag="s2f", name="s2f")
            out_sb = [pp.tile([128, F], u8, tag=f"o{m}", name=f"o{m}")
                      for m in range(8)]
            scratch = pp.tile([128, 1], i8, tag="scr", name="scr")
            scratch2 = pp.tile([128, 1], i8, tag="scr2", name="scr2")

            # gpsimd: zero the warmup operand and the s1p borders (zero
            # borders feed the 3x3 conv; interiors are fully overwritten)
            nc.gpsimd.memset(dummy[:], 0.0)
            nc.gpsimd.memset(s1p[:, :, :, 0:1, :], 0.0)
            nc.gpsimd.memset(s1p[:, :, :, 29:30, :], 0.0)
            nc.gpsimd.memset(s1p[:, :, :, :, 0:1], 0.0)
            nc.gpsimd.memset(s1p[:, :, :, :, 29:32], 0.0)

            # input stream: x chunks on the sync HWDGE ring, PACED so at most
            # ~3 transfers are in flight (unpaced, the 16-way packet
            # round-robin scrambles completion order and chunk k can land
            # 5us late).  The pacer is a throwaway 1-byte SBUF->SBUF dma
            # whose tile-inserted sem wait stalls the sync queue until
            # chunk k-3 has fully landed.
            nc.sync.dma_start(xi[0][:], x_d[0])
            nc.sync.dma_start(xi[1][:], x_d[1])
            for k in range(2, 8):
                nc.sync.dma_start(scratch[:], xi[k - 2][:, 0:1])
                nc.sync.dma_start(xi[k][:], x_d[k])
            # w1 split so kt0-2's slice lands before the first stage-1
            # matmul; w1b paced behind chunk2, w2/w3 deferred behind chunk5
            # (gpsimd wedge) so they don't steal HBM bandwidth from x
            nc.scalar.dma_start(w1_sb[:, 0:6], w1_d[:, 0:6])
            nc.scalar.dma_start(vec_sb[:], vec_d[:])
            nc.scalar.dma_start(scratch2[:], xi[2][:, 0:1])
            nc.scalar.dma_start(w1_sb[:, 6:16], w1_d[:, 6:16])
            nc.gpsimd.tensor_copy(scratch2[:], xi[5][:, 0:1])
            nc.gpsimd.dma_start(w2_sb[:], w2_d[:])
            nc.gpsimd.dma_start(w3_sb[:], w3_d[:])

            # i8 -> bf16 converts on DVE/ACT, image-0 halves first (they
            # gate stage-1 pass A; image-1 halves are for pass B)
            def convert(k, h, eng):
                lo, hi = h * HW_, (h + 1) * HW_
                if eng == "dve":
                    nc.vector.tensor_copy(x_sb[k][:, lo:hi], xi[k][:, lo:hi])
                else:
                    nc.scalar.activation(x_sb[k][:, lo:hi], xi[k][:, lo:hi],
                                         Act.Identity)

            for k in range(8):
                convert(k, 0, CONV_A[k])
            for k in range(8):
                convert(k, 1, CONV_B[k])

            # per-channel scale/bias column views (a' = alpha*2^-12, b' = beta*2^q)
            a1 = [vec_sb[:, m:m + 1] for m in range(2)]
            b1 = [vec_sb[:, 2 + m:3 + m] for m in range(2)]
            a2 = [vec_sb[:, 4 + m:5 + m] for m in range(2)]
            b2 = [vec_sb[:, 6 + m:7 + m] for m in range(2)]
            a3 = [vec_sb[:, 8 + m:9 + m] for m in range(8)]
            b3 = [vec_sb[:, 16 + m:17 + m] for m in range(8)]

            # ---- PE warmup (HAM un-throttle) while chunk0 streams in ----
            wps = psB.tile([128, 2, 512], f32, tag="ps", name="wps")
            for _ in range(N_WARMUP):
                nc.tensor.matmul(wps[:, 0, 0:256], dummy[:, 0:128],
                                 dummy[:, 0:256], start=True, stop=True)

            # ---- stage 1: bf16 1x1 conv, image-0 pass then image-1 pass ----
            def s1_pass(i):
                ps = [psA.tile([128, 2, 512], f32, tag="ps", name=f"ps1_{m}{i}")
                      for m in range(2)]
                for kt in range(8):
                    for m in range(2):
                        lhsT = w1_sb[:, kt * 2 + m]
                        for hb in range(2):
                            nc.tensor.matmul(
                                ps[m][:, hb, 0:FB], lhsT,
                                x_sb[kt][:, i * HW_ + hb * FB:
                                          i * HW_ + (hb + 1) * FB],
                                start=(kt == 0), stop=(kt == 7))
                return ps

            # stage-1/2 epilogue: A(m0) on DVE (u8 out, saturation = relu)
            # with DVE cast; A(m1) on ACT (relu -> i16) with DVE cast.
            def s12_epilogue(ps, i, a, b, dst, stage):
                r0 = sp.tile([128, 28, 28], u8, tag="r", name=f"r{stage}a_{i}")
                nc.vector.tensor_scalar(r0[:], ps[0][:, 0:2, 0:FB],
                                        a[0], b[0], Alu.mult, Alu.add)
                r1 = sp.tile([128, 28, 28], i16, tag="r", name=f"r{stage}b_{i}")
                nc.scalar.activation(r1[:], ps[1][:, 0:2, 0:FB],
                                     Act.Relu, bias=b[1], scale=a[1])
                if stage == 1:
                    nc.vector.tensor_copy(dst[:, 0, i, 1:29, 1:29], r0[:])
                    nc.vector.tensor_copy(dst[:, 1, i, 1:29, 1:29], r1[:])
                else:
                    nc.vector.tensor_copy(dst[:, 0, 2 * i:2 * i + 2, 0:FB], r0[:])
                    nc.vector.tensor_copy(dst[:, 1, 2 * i:2 * i + 2, 0:FB], r1[:])

            ps1_0 = s1_pass(0)
            s12_epilogue(ps1_0, 0, a1, b1, s1p, 1)
            ps1_1 = s1_pass(1)
            s12_epilogue(ps1_1, 1, a1, b1, s1p, 1)

            # ---- stage 2: fp8 DoubleRow 3x3 conv ----
            def s2_taps(ps2, i, taps):
                for tap in taps:
                    dy, dx = tap // 3, tap % 3
                    for m in range(2):
                        lhsT = w2_sb[:, tap * 2 + m]
                        for hb in range(2):
                            h0 = hb * 14
                            rhs = s1p[:, :, i, h0 + dy:h0 + dy + 14, dx:dx + 28]
                            nc.tensor.matmul(
                                ps2[m][:, hb, 0:FB], lhsT, rhs,
                                start=(tap == 0), stop=(tap == 8), perf_mode=DR)

            # ---- stage 3 per-iteration: 2 matmuls + A (psum->i16), then a
            # separately-emitted B+C (r += x; min->u8).  Emission order sets
            # the per-engine FIFO order, so B/C can be deferred to let other
            # ops (e.g. the stage-2 epilogue) slot in at their data-arrival
            # position.
            def s3_mm_a(i, m, pool):
                t = i * 8 + m
                ps3 = pool.tile([128, 2, 512], f32, tag="ps", name=f"ps3_{m}{i}")
                for hb in range(2):
                    nc.tensor.matmul(ps3[:, hb, 0:FB], w3_sb[:, m],
                                     s2f[:, :, 2 * i + hb, 0:FB],
                                     start=True, stop=True, perf_mode=DR)
                r = sp3.tile([128, HW_], i16, tag="r3", name=f"r3_{m}{i}")
                if A_ENG[t] == "act":
                    nc.scalar.activation(r[:], ps3[:, 0:2, 0:FB],
                                         Act.Identity, bias=b3[m], scale=a3[m])
                else:
                    nc.vector.tensor_scalar(r[:], ps3[:, 0:2, 0:FB],
                                            a3[m], b3[m], Alu.mult, Alu.add)
                return r

            def s3_bc(i, m, r):
                # B: r += x in place (i16 + bf16, exact)
                xs = x_sb[m][:, i * HW_:(i + 1) * HW_]
                nc.vector.tensor_tensor(r[:], r[:], xs, Alu.add)
                # C: out = min(r, 127) -> u8 (saturation clamps below at 0)
                dst = out_sb[m][:, i * HW_:(i + 1) * HW_]
                nc.vector.tensor_scalar(dst, r[:], 127.0, None, Alu.min)
                if i == 1:
                    nc.sync.dma_start(out_d[m], out_sb[m][:])

            # stage-2 image 0 (all taps), epilogue
            ps2_0 = [psA.tile([128, 2, 512], f32, tag="ps", name=f"ps2_{m}0")
                     for m in range(2)]
            s2_taps(ps2_0, 0, range(9))
            s12_epilogue(ps2_0, 0, a2, b2, s2f, 2)

            # stage-2 image 1 interleaved with stage-3 image 0.  B/C for
            # m4-7 are emitted AFTER the stage-2-i1 epilogue so the DVE
            # FIFO reaches the (critical-path) s2f-i1 ops at data arrival.
            ps2_1 = [psA.tile([128, 2, 512], f32, tag="ps", name=f"ps2_{m}1")
                     for m in range(2)]
            s2_taps(ps2_1, 1, range(0, 3))
            rs0 = [s3_mm_a(0, m, psB) for m in range(4)]
            for m in range(4):
                s3_bc(0, m, rs0[m])
            s2_taps(ps2_1, 1, range(3, 6))
            rs1 = [s3_mm_a(0, m, psB) for m in range(4, 8)]
            s2_taps(ps2_1, 1, range(6, 9))
            s12_epilogue(ps2_1, 1, a2, b2, s2f, 2)
            for m in range(4, 8):
                s3_bc(0, m, rs1[m - 4])

            # stage-3 image 1 (psum alternates psB / psA slots)
            for m in range(8):
                r = s3_mm_a(1, m, psB if m % 2 == 0 else psA)
                s3_bc(1, m, r)

    nc.compile()
    return nc


def _get_nc():
    if "nc" not in _CACHE:
        _CACHE["nc"] = _build()
    return _CACHE["nc"]


def _pack_inputs(inputs):
    """Host-side: effective weights, per-core shards, dtype casts."""
    f32 = np.float32
    x = np.asarray(inputs["x"])

    def eff(w2, s):
        return (np.asarray(w2, dtype=f32) *
                np.exp2(np.asarray(s).astype(f32))).astype(f32)

    # stage 1 (bf16): w1[p, kt*2+m, j] = W1_eff[kt*128+p, m*128+j]
    w1e = eff(inputs["w2_1"], inputs["s1"])[:, :, 0, 0]          # [O=256, I=1024]
    w1 = np.ascontiguousarray(
        w1e.T.reshape(8, 128, 2, 128).transpose(1, 0, 2, 3)     # [p, kt, m, j]
        .reshape(128, 16, 128)).astype(BF16)
    # stage 2 (fp8 pairs): logical W[b][p][j] = W2_eff[tap][b*128+p, m*128+j]
    w2e = eff(inputs["w2_2"], inputs["s2"])                      # [O, I, 3, 3]
    taps = np.stack([w2e[:, :, dy, dx].T                         # [I, O]
                     for dy in range(3) for dx in range(3)])     # [9, I=256, O=256]
    t5 = taps.reshape(9, 2, 128, 2, 128)                         # [tap, b, p, m, j]
    # stage 3 (fp8 pairs): logical W[b][p][j] = W3_eff[b*128+p, m*128+j]
    w3e = eff(inputs["w2_3"], inputs["s3"])[:, :, 0, 0]          # [O=1024, I=256]
    t3 = w3e.T.reshape(2, 128, 8, 128)                           # [b, p, m, j]
    # DRSW: phys[p, grp, 2c+b] = W[b][p][127-c] (A/B interleave, cols reversed)
    w2 = np.ascontiguousarray(
        t5[..., ::-1]                                            # j -> c=127-j
        .transpose(2, 0, 3, 4, 1)                                # [p, tap, m, c, b]
        .reshape(128, 18, 2, 128)).astype(FP8)
    w3 = np.ascontiguousarray(
        t3[..., ::-1].transpose(1, 2, 3, 0)                      # [p, m, c, b]
        .reshape(128, 8, 2, 128)).astype(FP8)

    vec = np.zeros((128, 24), dtype=f32)
    scl = np.exp2(f32(-12.0))
    for m in range(2):
        sl = slice(m * 128, (m + 1) * 128)
        vec[:, m] = np.asarray(inputs["alpha1"], dtype=f32)[sl] * scl
        vec[:, 2 + m] = (np.asarray(inputs["beta1"], dtype=f32)[sl] *
                         np.exp2(np.asarray(inputs["q1"]).astype(f32)[sl]))
        vec[:, 4 + m] = np.asarray(inputs["alpha2"], dtype=f32)[sl] * scl
        vec[:, 6 + m] = (np.asarray(inputs["beta2"], dtype=f32)[sl] *
                         np.exp2(np.asarray(inputs["q2"]).astype(f32)[sl]))
    for m in range(8):
        sl = slice(m * 128, (m + 1) * 128)
        vec[:, 8 + m] = np.asarray(inputs["alpha3"], dtype=f32)[sl] * scl
        vec[:, 16 + m] = (np.asarray(inputs["beta3"], dtype=f32)[sl] *
                          np.exp2(np.asarray(inputs["q3"]).astype(f32)[sl]))

    in_maps = []
    for c in range(N_CORES):
        xc = x[c * N_PER_CORE:(c + 1) * N_PER_CORE]              # [2, 1024, 28, 28]
        xc = np.ascontiguousarray(
            xc.transpose(1, 0, 2, 3).reshape(8, 128, F)).astype(np.int8)
        in_maps.append({"x": xc, "w1": w1, "w2": w2, "w3": w3, "vec": vec})
    return in_maps


def _assemble(results):
    outs = []
    for c in range(N_CORES):
        o = results[c]["out"]                                    # [8,128,1568] u8
        o = o.reshape(1024, N_PER_CORE, 28, 28).transpose(1, 0, 2, 3)
        outs.append(o)
    return np.concatenate(outs, axis=0).astype(np.float32)


def _run(inputs, trace=False, **kwargs):
    from concourse.bass_utils import run_bass_kernel_spmd
    nc = _get_nc()
    in_maps = _pack_inputs(inputs)
    res = run_bass_kernel_spmd(nc, in_maps, list(range(N_CORES)),
                               trace=trace, **kwargs)
    return _assemble(res.results), res


def kernel(**inputs):
    out, _ = _run(inputs)
    return out


# revision 14
# speedup vs baseline: 1.0244x; 1.0244x over previous
"""Bottleneck residual block (1x1 -> 3x3 -> 1x1 conv + BN + residual) on 8 NeuronCores.

Strategy: pure data-parallel over the batch dim (16 images -> 2 per core).
All convs are exact-integer arithmetic in disguise (int8 activations x
small power-of-two int weights), so matmuls are exact in any float format
wide enough: stage 1 runs bf16; stages 2+3 run fp8e4m3 with DoubleRow
(inner activations <= ~14 int, weights in {-4..4} are e4m3-exact).

v2 structure (vs the 60.5us baseline):
  - x is DMA'd as int8 (1.57MB not 3.2MB) and converted i8->bf16 on the
    otherwise-idle DVE/ACT/GPS engines during the head.
  - out is u8 (relu comes free in the fp32->u8 saturating convert).
  - stage 1 runs image-0 first (pass A) then image-1, so the stage-1
    epilogue for image 0 hides entirely under pass B's matmuls.
  - stage-3 image-0 matmuls are interleaved into stage-2 image-1's tap
    stream so their ACT/DVE/GPS epilogues overlap stage-2 PE time.
  - epilogues are spread across ACT+DVE+GPS via static engine tables:
      stage1/2: A = relu(rne(a*psum+b)) as ACT->i16 or DVE->u8(sat),
                cast to fp8 on the other of DVE/GPS.
      stage3:   A = rne(a*psum+b)->i16 (ACT/DVE/GPS tensor_scalar),
                B = r + x (DVE tensor_tensor, i16+bf16, exact),
                C = min(r2,127)->u8 (saturating convert clamps at 0).
  - PSUM: two pools of 2x[128,2,512]: psA holds the long-lived stage-1/2
    accumulators, psB cycles warmup + stage-3 tiles (stage-3 image-1 also
    borrows psA slots once stage 2 is done).

Shapes hardcoded for N=16, Cin=Cout=1024, width=256, H=W=28.
"""

import numpy as np
import ml_dtypes

BF16 = ml_dtypes.bfloat16
FP8 = ml_dtypes.float8_e4m3

N_CORES = 8
N_PER_CORE = 2          # images per core
HW_ = 28 * 28           # 784 spatial positions per image
F = N_PER_CORE * HW_    # 1568 free-dim elements per core
FB = 392                # matmul free-dim block (14 rows of 28)

N_WARMUP = 24           # dummy matmuls until the first x transfer lands

# x-chunk i8->bf16 conversion engine per chunk, in kt order.  GPSIMD
# ALU/copy ops are ~5-30x slower than DVE and poison concurrent DVE
# throughput - never use it for per-element work.
CONV = ["dve", "act", "dve", "act", "dve", "act", "dve", "act"]

# stage-3 epilogue engine tables, iter t = i*8 + m
# (GPSIMD cannot read PSUM; A is ACT/DVE only, B/C are DVE)
A_ENG = ["act"] * 16
B_ENG = ["dve"] * 16
C_ENG = ["dve"] * 16

_CACHE = {}


def _build():
    """Build + compile the per-core Bass kernel once per process."""
    import concourse.bacc as bacc
    import concourse.mybir as mybir
    import concourse.tile as tile

    dt = mybir.dt
    f32, bf16, i16, i8, u8, fp8 = (dt.float32, dt.bfloat16, dt.int16,
                                   dt.int8, dt.uint8, dt.float8e4)
    Alu = mybir.AluOpType
    Act = mybir.ActivationFunctionType
    DR = mybir.MatmulPerfMode.DoubleRowSwInterleave

    nc = bacc.Bacc("TRN2", target_bir_lowering=False, debug=False,
                   num_devices=N_CORES, enable_partition_id=False)

    x_d = nc.dram_tensor("x", [8, 128, F], i8, kind="ExternalInput")
    w1_d = nc.dram_tensor("w1", [128, 16, 128], bf16, kind="ExternalInput")
    w2_d = nc.dram_tensor("w2", [128, 18, 2, 128], fp8, kind="ExternalInput")
    w3_d = nc.dram_tensor("w3", [128, 8, 2, 128], fp8, kind="ExternalInput")
    vec_d = nc.dram_tensor("vec", [128, 24], f32, kind="ExternalInput")
    out_d = nc.dram_tensor("out", [8, 128, F], u8, kind="ExternalOutput")

    with tile.TileContext(nc) as tc:
        with (
            tc.tile_pool(name="persist", bufs=1) as pp,
            tc.tile_pool(name="stage", bufs=4) as sp,
            tc.tile_pool(name="stage3", bufs=8) as sp3,
            tc.tile_pool(name="psA", bufs=2, space="PSUM") as psA,
            tc.tile_pool(name="psB", bufs=2, space="PSUM") as psB,
        ):
            # ---- persistent SBUF tensors ----
            dummy = pp.tile([128, 256], bf16, tag="dummy", name="dummy")
            xi = [pp.tile([128, F], i8, tag=f"xi{k}", name=f"xi{k}")
                  for k in range(8)]
            x_sb = [pp.tile([128, F], bf16, tag=f"x{k}", name=f"x{k}")
                    for k in range(8)]
            w1_sb = pp.tile([128, 16, 128], bf16, tag="w1", name="w1")
            w2_sb = pp.tile([128, 18, 2, 128], fp8, tag="w2", name="w2")
            w3_sb = pp.tile([128, 8, 2, 128], fp8, tag="w3", name="w3")
            vec_sb = pp.tile([128, 24], f32, tag="vec", name="vec")
            s1p = pp.tile([128, 2, 2, 30, 32], fp8, tag="s1p", name="s1p")
            s2f = pp.tile([128, 2, 4, 400], fp8, tag="s2f", name="s2f")
            out_sb = [pp.tile([128, F], u8, tag=f"o{m}", name=f"o{m}")
                      for m in range(8)]
            scratch = pp.tile([128, 1], i8, tag="scr", name="scr")
            scratch2 = pp.tile([128, 1], i8, tag="scr2", name="scr2")

            # gpsimd: zero the warmup operand and the s1p borders (zero
            # borders feed the 3x3 conv; interiors are fully overwritten)
            nc.gpsimd.memset(dummy[:], 0.0)
            nc.gpsimd.memset(s1p[:, :, :, 0:1, :], 0.0)
            nc.gpsimd.memset(s1p[:, :, :, 29:30, :], 0.0)
            nc.gpsimd.memset(s1p[:, :, :, :, 0:1], 0.0)
            nc.gpsimd.memset(s1p[:, :, :, :, 29:32], 0.0)

            # input stream: x as three large transfers (big transfers avoid
            # the per-transfer ~2us completion receipt dominating and the
            # multi-transfer packet-round-robin completion scramble).
            # w1/vec in parallel on the scalar HWDGE ring; w2/w3 deferred
            # behind the second x transfer (gpsimd wedge).
            nc.sync.dma_start(xi[0][:], x_d[0])
            nc.sync.dma_start(xi[1][:], x_d[1])
            nc.sync.dma_start(xi[2][:], x_d[2])
            nc.sync.dma_start(xi[3][:], x_d[3])
            nc.sync.dma_start(xi[4][:], x_d[4])
            nc.sync.dma_start(xi[5][:], x_d[5])
            nc.sync.dma_start(xi[6][:], x_d[6])
            nc.sync.dma_start(xi[7][:], x_d[7])
            nc.scalar.dma_start(w1_sb[:], w1_d[:])
            nc.scalar.dma_start(vec_sb[:], vec_d[:])
            nc.gpsimd.tensor_copy(scratch2[:], xi[5][:, 0:1])
            nc.gpsimd.dma_start(w2_sb[:], w2_d[:])
            nc.gpsimd.dma_start(w3_sb[:], w3_d[:])

            # i8 -> bf16 full-chunk converts on DVE/ACT in kt order
            def convert(k):
                if CONV[k] == "dve":
                    nc.vector.tensor_copy(x_sb[k][:], xi[k][:])
                else:
                    nc.scalar.activation(x_sb[k][:], xi[k][:], Act.Identity)

            for k in range(8):
                convert(k)

            # per-channel scale/bias column views (a' = alpha*2^-12, b' = beta*2^q)
            a1 = [vec_sb[:, m:m + 1] for m in range(2)]
            b1 = [vec_sb[:, 2 + m:3 + m] for m in range(2)]
            a2 = [vec_sb[:, 4 + m:5 + m] for m in range(2)]
            b2 = [vec_sb[:, 6 + m:7 + m] for m in range(2)]
            a3 = [vec_sb[:, 8 + m:9 + m] for m in range(8)]
            b3 = [vec_sb[:, 16 + m:17 + m] for m in range(8)]

            # ---- PE warmup (HAM un-throttle) while chunk0 streams in ----
            wps = psB.tile([128, 2, 512], f32, tag="ps", name="wps")
            for _ in range(N_WARMUP):
                nc.tensor.matmul(wps[:, 0, 0:256], dummy[:, 0:128],
                                 dummy[:, 0:256], start=True, stop=True)

            # ---- stage 1: bf16 1x1 conv, image-0 pass then image-1 pass ----
            def s1_pass(i):
                ps = [psA.tile([128, 2, 512], f32, tag="ps", name=f"ps1_{m}{i}")
                      for m in range(2)]
                for kt in range(8):
                    for m in range(2):
                        lhsT = w1_sb[:, kt * 2 + m]
                        for hb in range(2):
                            nc.tensor.matmul(
                                ps[m][:, hb, 0:FB], lhsT,
                                x_sb[kt][:, i * HW_ + hb * FB:
                                          i * HW_ + (hb + 1) * FB],
                                start=(kt == 0), stop=(kt == 7))
                return ps

            # stage-1/2 epilogue: A(m0) on DVE (u8 out, saturation = relu)
            # with DVE cast; A(m1) on ACT (relu -> i16) with DVE cast.
            def s12_epilogue(ps, i, a, b, dst, stage):
                r0 = sp.tile([128, 28, 28], u8, tag="r", name=f"r{stage}a_{i}")
                nc.vector.tensor_scalar(r0[:], ps[0][:, 0:2, 0:FB],
                                        a[0], b[0], Alu.mult, Alu.add)
                r1 = sp.tile([128, 28, 28], i16, tag="r", name=f"r{stage}b_{i}")
                nc.scalar.activation(r1[:], ps[1][:, 0:2, 0:FB],
                                     Act.Relu, bias=b[1], scale=a[1])
                if stage == 1:
                    nc.vector.tensor_copy(dst[:, 0, i, 1:29, 1:29], r0[:])
                    nc.vector.tensor_copy(dst[:, 1, i, 1:29, 1:29], r1[:])
                else:
                    nc.vector.tensor_copy(dst[:, 0, 2 * i:2 * i + 2, 0:FB], r0[:])
                    nc.vector.tensor_copy(dst[:, 1, 2 * i:2 * i + 2, 0:FB], r1[:])

            ps1_0 = s1_pass(0)
            s12_epilogue(ps1_0, 0, a1, b1, s1p, 1)
            ps1_1 = s1_pass(1)
            s12_epilogue(ps1_1, 1, a1, b1, s1p, 1)

            # ---- stage 2: fp8 DoubleRow 3x3 conv ----
            def s2_taps(ps2, i, taps):
                for tap in taps:
                    dy, dx = tap // 3, tap % 3
                    for m in range(2):
                        lhsT = w2_sb[:, tap * 2 + m]
                        for hb in range(2):
                            h0 = hb * 14
                            rhs = s1p[:, :, i, h0 + dy:h0 + dy + 14, dx:dx + 28]
                            nc.tensor.matmul(
                                ps2[m][:, hb, 0:FB], lhsT, rhs,
                                start=(tap == 0), stop=(tap == 8), perf_mode=DR)

            # ---- stage 3 per-iteration: 2 matmuls + A (psum->i16), then a
            # separately-emitted B+C (r += x; min->u8).  Emission order sets
            # the per-engine FIFO order, so B/C can be deferred to let other
            # ops (e.g. the stage-2 epilogue) slot in at their data-arrival
            # position.
            def s3_mm_a(i, m, pool):
                t = i * 8 + m
                ps3 = pool.tile([128, 2, 512], f32, tag="ps", name=f"ps3_{m}{i}")
                for hb in range(2):
                    nc.tensor.matmul(ps3[:, hb, 0:FB], w3_sb[:, m],
                                     s2f[:, :, 2 * i + hb, 0:FB],
                                     start=True, stop=True, perf_mode=DR)
                r = sp3.tile([128, HW_], i16, tag="r3", name=f"r3_{m}{i}")
                if A_ENG[t] == "act":
                    nc.scalar.activation(r[:], ps3[:, 0:2, 0:FB],
                                         Act.Identity, bias=b3[m], scale=a3[m])
                else:
                    nc.vector.tensor_scalar(r[:], ps3[:, 0:2, 0:FB],
                                            a3[m], b3[m], Alu.mult, Alu.add)
                return r

            def s3_bc(i, m, r):
                # B: r += x in place (i16 + bf16, exact)
                xs = x_sb[m][:, i * HW_:(i + 1) * HW_]
                nc.vector.tensor_tensor(r[:], r[:], xs, Alu.add)
                # C: out = min(r, 127) -> u8 (saturation clamps below at 0)
                dst = out_sb[m][:, i * HW_:(i + 1) * HW_]
                nc.vector.tensor_scalar(dst, r[:], 127.0, None, Alu.min)
                if i == 1:
                    nc.sync.dma_start(out_d[m], out_sb[m][:])

            # stage-2 image 0 (all taps), epilogue
            ps2_0 = [psA.tile([128, 2, 512], f32, tag="ps", name=f"ps2_{m}0")
                     for m in range(2)]
            s2_taps(ps2_0, 0, range(9))
            s12_epilogue(ps2_0, 0, a2, b2, s2f, 2)

            # stage-2 image 1 interleaved with stage-3 image 0.  B/C for
            # m4-7 are emitted AFTER the stage-2-i1 epilogue so the DVE
            # FIFO reaches the (critical-path) s2f-i1 ops at data arrival.
            ps2_1 = [psA.tile([128, 2, 512], f32, tag="ps", name=f"ps2_{m}1")
                     for m in range(2)]
            s2_taps(ps2_1, 1, range(0, 3))
            rs0 = [s3_mm_a(0, m, psB) for m in range(4)]
            for m in range(4):
                s3_bc(0, m, rs0[m])
            s2_taps(ps2_1, 1, range(3, 6))
            rs1 = [s3_mm_a(0, m, psB) for m in range(4, 8)]
            s2_taps(ps2_1, 1, range(6, 9))
            s12_epilogue(ps2_1, 1, a2, b2, s2f, 2)
            for m in range(4, 8):
                s3_bc(0, m, rs1[m - 4])

            # stage-3 image 1 (psum alternates psB / psA slots)
            for m in range(8):
                r = s3_mm_a(1, m, psB if m % 2 == 0 else psA)
                s3_bc(1, m, r)

    nc.compile()
    return nc


def _get_nc():
    if "nc" not in _CACHE:
        _CACHE["nc"] = _build()
    return _CACHE["nc"]


def _pack_inputs(inputs):
    """Host-side: effective weights, per-core shards, dtype casts."""
    f32 = np.float32
    x = np.asarray(inputs["x"])

    def eff(w2, s):
        return (np.asarray(w2, dtype=f32) *
                np.exp2(np.asarray(s).astype(f32))).astype(f32)

    # stage 1 (bf16): w1[p, kt*2+m, j] = W1_eff[kt*128+p, m*128+j]
    w1e = eff(inputs["w2_1"], inputs["s1"])[:, :, 0, 0]          # [O=256, I=1024]
    w1 = np.ascontiguousarray(
        w1e.T.reshape(8, 128, 2, 128).transpose(1, 0, 2, 3)     # [p, kt, m, j]
        .reshape(128, 16, 128)).astype(BF16)
    # stage 2 (fp8 pairs): logical W[b][p][j] = W2_eff[tap][b*128+p, m*128+j]
    w2e = eff(inputs["w2_2"], inputs["s2"])                      # [O, I, 3, 3]
    taps = np.stack([w2e[:, :, dy, dx].T                         # [I, O]
                     for dy in range(3) for dx in range(3)])     # [9, I=256, O=256]
    t5 = taps.reshape(9, 2, 128, 2, 128)                         # [tap, b, p, m, j]
    # stage 3 (fp8 pairs): logical W[b][p][j] = W3_eff[b*128+p, m*128+j]
    w3e = eff(inputs["w2_3"], inputs["s3"])[:, :, 0, 0]          # [O=1024, I=256]
    t3 = w3e.T.reshape(2, 128, 8, 128)                           # [b, p, m, j]
    # DRSW: phys[p, grp, 2c+b] = W[b][p][127-c] (A/B interleave, cols reversed)
    w2 = np.ascontiguousarray(
        t5[..., ::-1]                                            # j -> c=127-j
        .transpose(2, 0, 3, 4, 1)                                # [p, tap, m, c, b]
        .reshape(128, 18, 2, 128)).astype(FP8)
    w3 = np.ascontiguousarray(
        t3[..., ::-1].transpose(1, 2, 3, 0)                      # [p, m, c, b]
        .reshape(128, 8, 2, 128)).astype(FP8)

    vec = np.zeros((128, 24), dtype=f32)
    scl = np.exp2(f32(-12.0))
    for m in range(2):
        sl = slice(m * 128, (m + 1) * 128)
        vec[:, m] = np.asarray(inputs["alpha1"], dtype=f32)[sl] * scl
        vec[:, 2 + m] = (np.asarray(inputs["beta1"], dtype=f32)[sl] *
                         np.exp2(np.asarray(inputs["q1"]).astype(f32)[sl]))
        vec[:, 4 + m] = np.asarray(inputs["alpha2"], dtype=f32)[sl] * scl
        vec[:, 6 + m] = (np.asarray(inputs["beta2"], dtype=f32)[sl] *
                         np.exp2(np.asarray(inputs["q2"]).astype(f32)[sl]))
    for m in range(8):
        sl = slice(m * 128, (m + 1) * 128)
        vec[:, 8 + m] = np.asarray(inputs["alpha3"], dtype=f32)[sl] * scl
        vec[:, 16 + m] = (np.asarray(inputs["beta3"], dtype=f32)[sl] *
                          np.exp2(np.asarray(inputs["q3"]).astype(f32)[sl]))

    in_maps = []
    for c in range(N_CORES):
        xc = x[c * N_PER_CORE:(c + 1) * N_PER_CORE]              # [2, 1024, 28, 28]
        xc = np.ascontiguousarray(
            xc.transpose(1, 0, 2, 3).reshape(8, 128, F)).astype(np.int8)
        in_maps.append({"x": xc, "w1": w1, "w2": w2, "w3": w3, "vec": vec})
    return in_maps


def _assemble(results):
    outs = []
    for c in range(N_CORES):
        o = results[c]["out"]                                    # [8,128,1568] u8
        o = o.reshape(1024, N_PER_CORE, 28, 28).transpose(1, 0, 2, 3)
        outs.append(o)
    return np.concatenate(outs, axis=0).astype(np.float32)


def _run(inputs, trace=False, **kwargs):
    from concourse.bass_utils import run_bass_kernel_spmd
    nc = _get_nc()
    in_maps = _pack_inputs(inputs)
    res = run_bass_kernel_spmd(nc, in_maps, list(range(N_CORES)),
                               trace=trace, **kwargs)
    return _assemble(res.results), res


def kernel(**inputs):
    out, _ = _run(inputs)
    return out


# revision 19
# speedup vs baseline: 1.0469x; 1.0219x over previous
"""Bottleneck residual block (1x1 -> 3x3 -> 1x1 conv + BN + residual) on 8 NeuronCores.

Strategy: pure data-parallel over the batch dim (16 images -> 2 per core).
All convs are exact-integer arithmetic in disguise (int8 activations x
small power-of-two int weights), so matmuls are exact in any float format
wide enough: stage 1 runs bf16; stages 2+3 run fp8e4m3 with DoubleRow
(inner activations <= ~14 int, weights in {-4..4} are e4m3-exact).

v2 structure (vs the 60.5us baseline):
  - x is DMA'd as int8 (1.57MB not 3.2MB) and converted i8->bf16 on the
    otherwise-idle DVE/ACT/GPS engines during the head.
  - out is u8 (relu comes free in the fp32->u8 saturating convert).
  - stage 1 runs image-0 first (pass A) then image-1, so the stage-1
    epilogue for image 0 hides entirely under pass B's matmuls.
  - stage-3 image-0 matmuls are interleaved into stage-2 image-1's tap
    stream so their ACT/DVE/GPS epilogues overlap stage-2 PE time.
  - epilogues are spread across ACT+DVE+GPS via static engine tables:
      stage1/2: A = relu(rne(a*psum+b)) as ACT->i16 or DVE->u8(sat),
                cast to fp8 on the other of DVE/GPS.
      stage3:   A = rne(a*psum+b)->i16 (ACT/DVE/GPS tensor_scalar),
                B = r + x (DVE tensor_tensor, i16+bf16, exact),
                C = min(r2,127)->u8 (saturating convert clamps at 0).
  - PSUM: two pools of 2x[128,2,512]: psA holds the long-lived stage-1/2
    accumulators, psB cycles warmup + stage-3 tiles (stage-3 image-1 also
    borrows psA slots once stage 2 is done).

Shapes hardcoded for N=16, Cin=Cout=1024, width=256, H=W=28.
"""

import numpy as np
import ml_dtypes

BF16 = ml_dtypes.bfloat16
FP8 = ml_dtypes.float8_e4m3

N_CORES = 8
N_PER_CORE = 2          # images per core
HW_ = 28 * 28           # 784 spatial positions per image
F = N_PER_CORE * HW_    # 1568 free-dim elements per core
FB = 392                # matmul free-dim block (14 rows of 28)

N_WARMUP = 17           # dummy matmuls until the first x transfer lands

# x-chunk i8->bf16 conversion engine per chunk, in kt order.  GPSIMD
# ALU/copy ops are ~5-30x slower than DVE and poison concurrent DVE
# throughput - never use it for per-element work.
CONV = ["dve", "act", "dve", "act", "dve", "act", "dve", "dve"]

# stage-3 epilogue engine tables, iter t = i*8 + m
# (GPSIMD cannot read PSUM; A is ACT/DVE only, B/C are DVE)
A_ENG = ["act"] * 16
B_ENG = ["dve"] * 16
C_ENG = ["dve"] * 16

_CACHE = {}


def _build():
    """Build + compile the per-core Bass kernel once per process."""
    import concourse.bacc as bacc
    import concourse.mybir as mybir
    import concourse.tile as tile

    dt = mybir.dt
    f32, bf16, i16, i8, u8, fp8 = (dt.float32, dt.bfloat16, dt.int16,
                                   dt.int8, dt.uint8, dt.float8e4)
    Alu = mybir.AluOpType
    Act = mybir.ActivationFunctionType
    DR = mybir.MatmulPerfMode.DoubleRowSwInterleave

    nc = bacc.Bacc("TRN2", target_bir_lowering=False, debug=False,
                   num_devices=N_CORES, enable_partition_id=False)

    x_d = nc.dram_tensor("x", [128, 8, F], i8, kind="ExternalInput")
    w1_d = nc.dram_tensor("w1", [128, 16, 128], bf16, kind="ExternalInput")
    w2_d = nc.dram_tensor("w2", [128, 18, 2, 128], fp8, kind="ExternalInput")
    w3_d = nc.dram_tensor("w3", [128, 8, 2, 128], fp8, kind="ExternalInput")
    vec_d = nc.dram_tensor("vec", [128, 24], f32, kind="ExternalInput")
    out_d = nc.dram_tensor("out", [8, 128, F], u8, kind="ExternalOutput")

    with tile.TileContext(nc) as tc:
        with (
            tc.tile_pool(name="persist", bufs=1) as pp,
            tc.tile_pool(name="stage", bufs=4) as sp,
            tc.tile_pool(name="stage3", bufs=8) as sp3,
            tc.tile_pool(name="psA", bufs=2, space="PSUM") as psA,
            tc.tile_pool(name="psB", bufs=2, space="PSUM") as psB,
        ):
            # ---- persistent SBUF tensors ----
            dummy = pp.tile([128, 256], bf16, tag="dummy", name="dummy")
            xi = pp.tile([128, 8, F], i8, tag="xi", name="xi")
            x_sb = [pp.tile([128, F], bf16, tag=f"x{k}", name=f"x{k}")
                    for k in range(8)]
            w1_sb = pp.tile([128, 16, 128], bf16, tag="w1", name="w1")
            w2_sb = pp.tile([128, 18, 2, 128], fp8, tag="w2", name="w2")
            w3_sb = pp.tile([128, 8, 2, 128], fp8, tag="w3", name="w3")
            vec_sb = pp.tile([128, 24], f32, tag="vec", name="vec")
            s1p = pp.tile([128, 2, 2, 30, 32], fp8, tag="s1p", name="s1p")
            s2f = pp.tile([128, 2, 4, 400], fp8, tag="s2f", name="s2f")
            out_sb = [pp.tile([128, F], u8, tag=f"o{m}", name=f"o{m}")
                      for m in range(8)]
            scratch = pp.tile([128, 1], i8, tag="scr", name="scr")
            scratch2 = pp.tile([128, 1], i8, tag="scr2", name="scr2")

            # gpsimd: zero the warmup operand and the s1p borders (zero
            # borders feed the 3x3 conv; interiors are fully overwritten)
            nc.gpsimd.memset(dummy[:], 0.0)
            nc.gpsimd.memset(s1p[:, :, :, 0:1, :], 0.0)
            nc.gpsimd.memset(s1p[:, :, :, 29:30, :], 0.0)
            nc.gpsimd.memset(s1p[:, :, :, :, 0:1], 0.0)
            nc.gpsimd.memset(s1p[:, :, :, :, 29:32], 0.0)

            # input stream: x as three large transfers with 3-6KB
            # per-partition rows (small rows run the DMA at <150GB/s; big
            # rows near line rate, and few transfers avoid the
            # packet-round-robin completion scramble).  w1/vec in parallel
            # on the scalar HWDGE ring; w2/w3 deferred behind the last x
            # transfer (gpsimd wedge).
            nc.sync.dma_start(xi[:, 0:2], x_d[:, 0:2])
            nc.sync.dma_start(xi[:, 2:5], x_d[:, 2:5])
            nc.sync.dma_start(xi[:, 5:8], x_d[:, 5:8])
            nc.scalar.dma_start(w1_sb[:], w1_d[:])
            nc.scalar.dma_start(vec_sb[:], vec_d[:])
            nc.gpsimd.tensor_copy(scratch2[:], xi[:, 7, 0:1])
            nc.gpsimd.dma_start(w2_sb[:], w2_d[:])
            nc.gpsimd.dma_start(w3_sb[:], w3_d[:])

            # i8 -> bf16 full-chunk converts on DVE/ACT in kt order
            def convert(k):
                if CONV[k] == "dve":
                    nc.vector.tensor_copy(x_sb[k][:], xi[:, k])
                else:
                    nc.scalar.activation(x_sb[k][:], xi[:, k], Act.Identity)

            for k in range(8):
                convert(k)

            # per-channel scale/bias column views (a' = alpha*2^-12, b' = beta*2^q)
            a1 = [vec_sb[:, m:m + 1] for m in range(2)]
            b1 = [vec_sb[:, 2 + m:3 + m] for m in range(2)]
            a2 = [vec_sb[:, 4 + m:5 + m] for m in range(2)]
            b2 = [vec_sb[:, 6 + m:7 + m] for m in range(2)]
            a3 = [vec_sb[:, 8 + m:9 + m] for m in range(8)]
            b3 = [vec_sb[:, 16 + m:17 + m] for m in range(8)]

            # ---- PE warmup (HAM un-throttle) while chunk0 streams in ----
            wps = psB.tile([128, 2, 512], f32, tag="ps", name="wps")
            for _ in range(N_WARMUP):
                nc.tensor.matmul(wps[:, 0, 0:256], dummy[:, 0:128],
                                 dummy[:, 0:256], start=True, stop=True)

            # ---- stage 1: bf16 1x1 conv, image-0 pass then image-1 pass ----
            def s1_pass(i):
                ps = [psA.tile([128, 2, 512], f32, tag="ps", name=f"ps1_{m}{i}")
                      for m in range(2)]
                for kt in range(8):
                    for m in range(2):
                        lhsT = w1_sb[:, kt * 2 + m]
                        for hb in range(2):
                            nc.tensor.matmul(
                                ps[m][:, hb, 0:FB], lhsT,
                                x_sb[kt][:, i * HW_ + hb * FB:
                                          i * HW_ + (hb + 1) * FB],
                                start=(kt == 0), stop=(kt == 7))
                return ps

            # stage-1/2 epilogue: A(m0) on DVE (u8 out, saturation = relu)
            # with DVE cast; A(m1) on ACT (relu -> i16) with DVE cast.
            def s12_epilogue(ps, i, a, b, dst, stage):
                r0 = sp.tile([128, 28, 28], u8, tag="r", name=f"r{stage}a_{i}")
                nc.vector.tensor_scalar(r0[:], ps[0][:, 0:2, 0:FB],
                                        a[0], b[0], Alu.mult, Alu.add)
                r1 = sp.tile([128, 28, 28], i16, tag="r", name=f"r{stage}b_{i}")
                nc.scalar.activation(r1[:], ps[1][:, 0:2, 0:FB],
                                     Act.Relu, bias=b[1], scale=a[1])
                if stage == 1:
                    nc.vector.tensor_copy(dst[:, 0, i, 1:29, 1:29], r0[:])
                    nc.vector.tensor_copy(dst[:, 1, i, 1:29, 1:29], r1[:])
                else:
                    nc.vector.tensor_copy(dst[:, 0, 2 * i:2 * i + 2, 0:FB], r0[:])
                    nc.vector.tensor_copy(dst[:, 1, 2 * i:2 * i + 2, 0:FB], r1[:])

            ps1_0 = s1_pass(0)
            s12_epilogue(ps1_0, 0, a1, b1, s1p, 1)
            ps1_1 = s1_pass(1)
            s12_epilogue(ps1_1, 1, a1, b1, s1p, 1)

            # ---- stage 2: fp8 DoubleRow 3x3 conv ----
            def s2_taps(ps2, i, taps):
                for tap in taps:
                    dy, dx = tap // 3, tap % 3
                    for m in range(2):
                        lhsT = w2_sb[:, tap * 2 + m]
                        for hb in range(2):
                            h0 = hb * 14
                            rhs = s1p[:, :, i, h0 + dy:h0 + dy + 14, dx:dx + 28]
                            nc.tensor.matmul(
                                ps2[m][:, hb, 0:FB], lhsT, rhs,
                                start=(tap == 0), stop=(tap == 8), perf_mode=DR)

            # ---- stage 3 per-iteration: 2 matmuls + A (psum->i16), then a
            # separately-emitted B+C (r += x; min->u8).  Emission order sets
            # the per-engine FIFO order, so B/C can be deferred to let other
            # ops (e.g. the stage-2 epilogue) slot in at their data-arrival
            # position.
            def s3_mm_a(i, m, pool):
                t = i * 8 + m
                ps3 = pool.tile([128, 2, 512], f32, tag="ps", name=f"ps3_{m}{i}")
                for hb in range(2):
                    nc.tensor.matmul(ps3[:, hb, 0:FB], w3_sb[:, m],
                                     s2f[:, :, 2 * i + hb, 0:FB],
                                     start=True, stop=True, perf_mode=DR)
                r = sp3.tile([128, HW_], i16, tag="r3", name=f"r3_{m}{i}")
                if A_ENG[t] == "act":
                    nc.scalar.activation(r[:], ps3[:, 0:2, 0:FB],
                                         Act.Identity, bias=b3[m], scale=a3[m])
                else:
                    nc.vector.tensor_scalar(r[:], ps3[:, 0:2, 0:FB],
                                            a3[m], b3[m], Alu.mult, Alu.add)
                return r

            def s3_bc(i, m, r):
                # B: r += x in place (i16 + bf16, exact)
                xs = x_sb[m][:, i * HW_:(i + 1) * HW_]
                nc.vector.tensor_tensor(r[:], r[:], xs, Alu.add)
                # C: out = min(r, 127) -> u8 (saturation clamps below at 0)
                dst = out_sb[m][:, i * HW_:(i + 1) * HW_]
                nc.vector.tensor_scalar(dst, r[:], 127.0, None, Alu.min)
                if i == 1:
                    nc.sync.dma_start(out_d[m], out_sb[m][:])

            # stage-2 image 0 (all taps), epilogue
            ps2_0 = [psA.tile([128, 2, 512], f32, tag="ps", name=f"ps2_{m}0")
                     for m in range(2)]
            s2_taps(ps2_0, 0, range(9))
            s12_epilogue(ps2_0, 0, a2, b2, s2f, 2)

            # stage-2 image 1 interleaved with stage-3 image 0.  B/C for
            # m4-7 are emitted AFTER the stage-2-i1 epilogue so the DVE
            # FIFO reaches the (critical-path) s2f-i1 ops at data arrival.
            ps2_1 = [psA.tile([128, 2, 512], f32, tag="ps", name=f"ps2_{m}1")
                     for m in range(2)]
            s2_taps(ps2_1, 1, range(0, 3))
            rs0 = [s3_mm_a(0, m, psB) for m in range(4)]
            for m in range(4):
                s3_bc(0, m, rs0[m])
            s2_taps(ps2_1, 1, range(3, 6))
            rs1 = [s3_mm_a(0, m, psB) for m in range(4, 8)]
            s2_taps(ps2_1, 1, range(6, 9))
            s12_epilogue(ps2_1, 1, a2, b2, s2f, 2)
            for m in range(4, 8):
                s3_bc(0, m, rs1[m - 4])

            # stage-3 image 1 (psum alternates psB / psA slots)
            for m in range(8):
                r = s3_mm_a(1, m, psB if m % 2 == 0 else psA)
                s3_bc(1, m, r)

    nc.compile()
    return nc


def _get_nc():
    if "nc" not in _CACHE:
        _CACHE["nc"] = _build()
    return _CACHE["nc"]


def _pack_inputs(inputs):
    """Host-side: effective weights, per-core shards, dtype casts."""
    f32 = np.float32
    x = np.asarray(inputs["x"])

    def eff(w2, s):
        return (np.asarray(w2, dtype=f32) *
                np.exp2(np.asarray(s).astype(f32))).astype(f32)

    # stage 1 (bf16): w1[p, kt*2+m, j] = W1_eff[kt*128+p, m*128+j]
    w1e = eff(inputs["w2_1"], inputs["s1"])[:, :, 0, 0]          # [O=256, I=1024]
    w1 = np.ascontiguousarray(
        w1e.T.reshape(8, 128, 2, 128).transpose(1, 0, 2, 3)     # [p, kt, m, j]
        .reshape(128, 16, 128)).astype(BF16)
    # stage 2 (fp8 pairs): logical W[b][p][j] = W2_eff[tap][b*128+p, m*128+j]
    w2e = eff(inputs["w2_2"], inputs["s2"])                      # [O, I, 3, 3]
    taps = np.stack([w2e[:, :, dy, dx].T                         # [I, O]
                     for dy in range(3) for dx in range(3)])     # [9, I=256, O=256]
    t5 = taps.reshape(9, 2, 128, 2, 128)                         # [tap, b, p, m, j]
    # stage 3 (fp8 pairs): logical W[b][p][j] = W3_eff[b*128+p, m*128+j]
    w3e = eff(inputs["w2_3"], inputs["s3"])[:, :, 0, 0]          # [O=1024, I=256]
    t3 = w3e.T.reshape(2, 128, 8, 128)                           # [b, p, m, j]
    # DRSW: phys[p, grp, 2c+b] = W[b][p][127-c] (A/B interleave, cols reversed)
    w2 = np.ascontiguousarray(
        t5[..., ::-1]                                            # j -> c=127-j
        .transpose(2, 0, 3, 4, 1)                                # [p, tap, m, c, b]
        .reshape(128, 18, 2, 128)).astype(FP8)
    w3 = np.ascontiguousarray(
        t3[..., ::-1].transpose(1, 2, 3, 0)                      # [p, m, c, b]
        .reshape(128, 8, 2, 128)).astype(FP8)

    vec = np.zeros((128, 24), dtype=f32)
    scl = np.exp2(f32(-12.0))
    for m in range(2):
        sl = slice(m * 128, (m + 1) * 128)
        vec[:, m] = np.asarray(inputs["alpha1"], dtype=f32)[sl] * scl
        vec[:, 2 + m] = (np.asarray(inputs["beta1"], dtype=f32)[sl] *
                         np.exp2(np.asarray(inputs["q1"]).astype(f32)[sl]))
        vec[:, 4 + m] = np.asarray(inputs["alpha2"], dtype=f32)[sl] * scl
        vec[:, 6 + m] = (np.asarray(inputs["beta2"], dtype=f32)[sl] *
                         np.exp2(np.asarray(inputs["q2"]).astype(f32)[sl]))
    for m in range(8):
        sl = slice(m * 128, (m + 1) * 128)
        vec[:, 8 + m] = np.asarray(inputs["alpha3"], dtype=f32)[sl] * scl
        vec[:, 16 + m] = (np.asarray(inputs["beta3"], dtype=f32)[sl] *
                          np.exp2(np.asarray(inputs["q3"]).astype(f32)[sl]))

    in_maps = []
    for c in range(N_CORES):
        xc = x[c * N_PER_CORE:(c + 1) * N_PER_CORE]              # [2, 1024, 28, 28]
        # -> [p, kt, img*28*28]: partition-major so the DMA rows are large
        xc = np.ascontiguousarray(
            xc.reshape(2, 8, 128, HW_).transpose(2, 1, 0, 3)
            .reshape(128, 8, F)).astype(np.int8)
        in_maps.append({"x": xc, "w1": w1, "w2": w2, "w3": w3, "vec": vec})
    return in_maps


def _assemble(results):
    outs = []
    for c in range(N_CORES):
        o = results[c]["out"]                                    # [8,128,1568] u8
        o = o.reshape(1024, N_PER_CORE, 28, 28).transpose(1, 0, 2, 3)
        outs.append(o)
    return np.concatenate(outs, axis=0).astype(np.float32)


def _run(inputs, trace=False, **kwargs):
    from concourse.bass_utils import run_bass_kernel_spmd
    nc = _get_nc()
    in_maps = _pack_inputs(inputs)
    res = run_bass_kernel_spmd(nc, in_maps, list(range(N_CORES)),
                               trace=trace, **kwargs)
    return _assemble(res.results), res


def kernel(**inputs):
    out, _ = _run(inputs)
    return out


# revision 20
# speedup vs baseline: 1.1019x; 1.0525x over previous
"""Bottleneck residual block (1x1 -> 3x3 -> 1x1 conv + BN + residual) on 8 NeuronCores.

Strategy: pure data-parallel over the batch dim (16 images -> 2 per core).
All convs are exact-integer arithmetic in disguise (int8 activations x
small power-of-two int weights), so matmuls are exact in any float format
wide enough: stage 1 runs bf16; stages 2+3 run fp8e4m3 with DoubleRow
(inner activations <= ~14 int, weights in {-4..4} are e4m3-exact).

v7 structure (baseline head + restructured tail):
  - head/stage-1 as the 60.5us baseline (bf16 x chunks streamed on sync,
    kt-interleaved consumption, PE warmup during DMA) - measured to be the
    optimal overlap of the 3.2MB x stream with stage-1's matmuls.
  - out is u8 (relu comes free in the fp32->u8 saturating convert; the
    final clamp is a single DVE min op; out DMA is half the bytes).
  - stage-3 image-0 matmuls interleave into stage-2 image-1's tap stream
    so half the stage-3 epilogue overlaps stage-2 PE time.
  - epilogue ops are balanced across ACT and DVE (GPSIMD is useless for
    per-element work and cannot read PSUM):
      stage1/2: A(m0) = DVE tensor_scalar -> u8 (saturation = relu),
                A(m1) = ACT Relu -> i16; casts to fp8 on ACT + DVE.
      stage3:   A = ACT Identity -> i16 (rne in convert);
                B = DVE r += x (i16+bf16, exact);
                C = DVE min(r,127) -> u8.
  - emission order is tuned so each engine's FIFO matches data-arrival
    order (stage-2-i1's epilogue ops sit before stage-3-i0's deferred B/C).
  - PSUM: two pools of 2x[128,2,512]: psA holds image-0 stage-1 + all
    stage-2 accumulators + odd stage-3-i1 tiles; psB holds warmup +
    image-1 stage-1 + stage-3-i0 + even stage-3-i1 tiles.

Shapes hardcoded for N=16, Cin=Cout=1024, width=256, H=W=28.
"""

import numpy as np
import ml_dtypes

BF16 = ml_dtypes.bfloat16
FP8 = ml_dtypes.float8_e4m3

N_CORES = 8
N_PER_CORE = 2          # images per core
HW_ = 28 * 28           # 784 spatial positions per image
F = N_PER_CORE * HW_    # 1568 free-dim elements per core
FB = 392                # matmul free-dim block (14 rows of 28)

N_WARMUP = 17           # dummy matmuls while the x chunks stream in

_CACHE = {}


def _build():
    """Build + compile the per-core Bass kernel once per process."""
    import concourse.bacc as bacc
    import concourse.mybir as mybir
    import concourse.tile as tile

    dt = mybir.dt
    f32, bf16, i16, u8, fp8 = (dt.float32, dt.bfloat16, dt.int16,
                               dt.uint8, dt.float8e4)
    Alu = mybir.AluOpType
    Act = mybir.ActivationFunctionType
    DR = mybir.MatmulPerfMode.DoubleRowSwInterleave

    nc = bacc.Bacc("TRN2", target_bir_lowering=False, debug=False,
                   num_devices=N_CORES, enable_partition_id=False)

    x_d = nc.dram_tensor("x", [8, 128, F], bf16, kind="ExternalInput")
    w1_d = nc.dram_tensor("w1", [128, 16, 128], bf16, kind="ExternalInput")
    w2_d = nc.dram_tensor("w2", [128, 18, 2, 128], fp8, kind="ExternalInput")
    w3_d = nc.dram_tensor("w3", [128, 8, 2, 128], fp8, kind="ExternalInput")
    vec_d = nc.dram_tensor("vec", [128, 24], f32, kind="ExternalInput")
    out_d = nc.dram_tensor("out", [8, 128, F], u8, kind="ExternalOutput")

    with tile.TileContext(nc) as tc:
        with (
            tc.tile_pool(name="persist", bufs=1) as pp,
            tc.tile_pool(name="stage", bufs=4) as sp,
            tc.tile_pool(name="stage3", bufs=8) as sp3,
            tc.tile_pool(name="psA", bufs=2, space="PSUM") as psA,
            tc.tile_pool(name="psB", bufs=2, space="PSUM") as psB,
        ):
            # ---- persistent SBUF tensors ----
            dummy = pp.tile([128, 256], bf16, tag="dummy", name="dummy")
            x_sb = [pp.tile([128, F], bf16, tag=f"x{k}", name=f"x{k}")
                    for k in range(8)]
            w1_sb = pp.tile([128, 16, 128], bf16, tag="w1", name="w1")
            w2_sb = pp.tile([128, 18, 2, 128], fp8, tag="w2", name="w2")
            w3_sb = pp.tile([128, 8, 2, 128], fp8, tag="w3", name="w3")
            vec_sb = pp.tile([128, 24], f32, tag="vec", name="vec")
            s1p = pp.tile([128, 2, 2, 30, 32], fp8, tag="s1p", name="s1p")
            s2f = pp.tile([128, 2, 4, 400], fp8, tag="s2f", name="s2f")
            out_sb = [pp.tile([128, F], u8, tag=f"o{m}", name=f"o{m}")
                      for m in range(8)]
            scratch = pp.tile([128, 1], bf16, tag="scr", name="scr")

            # gpsimd: zero the warmup operand and the s1p borders (zero
            # borders feed the 3x3 conv; interiors are fully overwritten)
            nc.gpsimd.memset(dummy[:], 0.0)
            nc.gpsimd.memset(s1p[:, :, :, 0:1, :], 0.0)
            nc.gpsimd.memset(s1p[:, :, :, 29:30, :], 0.0)
            nc.gpsimd.memset(s1p[:, :, :, :, 0:1], 0.0)
            nc.gpsimd.memset(s1p[:, :, :, :, 29:32], 0.0)

            # critical input stream: x chunks on sync in stage-1 consumption
            # order; w1 and vec from the scalar queue in parallel; w2/w3
            # deferred behind chunk 6 (gpsimd wedge) so their transfers
            # trail the x stream
            for k in range(8):
                nc.sync.dma_start(x_sb[k][:], x_d[k])
            nc.scalar.dma_start(w1_sb[:], w1_d[:])
            nc.scalar.dma_start(vec_sb[:], vec_d[:])
            nc.gpsimd.tensor_copy(scratch[:], x_sb[6][:, 0:1])
            nc.gpsimd.dma_start(w2_sb[:], w2_d[:])
            nc.gpsimd.dma_start(w3_sb[:], w3_d[:])

            # per-channel scale/bias column views (a' = alpha*2^-12, b' = beta*2^q)
            a1 = [vec_sb[:, m:m + 1] for m in range(2)]
            b1 = [vec_sb[:, 2 + m:3 + m] for m in range(2)]
            a2 = [vec_sb[:, 4 + m:5 + m] for m in range(2)]
            b2 = [vec_sb[:, 6 + m:7 + m] for m in range(2)]
            a3 = [vec_sb[:, 8 + m:9 + m] for m in range(8)]
            b3 = [vec_sb[:, 16 + m:17 + m] for m in range(8)]

            # ---- PE warmup (HAM un-throttle) while the x stream lands ----
            wps = psB.tile([128, 2, 512], f32, tag="ps", name="wps")
            for _ in range(N_WARMUP):
                nc.tensor.matmul(wps[:, 0, 0:256], dummy[:, 0:128],
                                 dummy[:, 0:256], start=True, stop=True)

            # ---- stage 1: bf16 1x1 conv (K=1024 -> M=256), kt-outer so the
            # matmuls consume x chunks in DMA arrival order.  Image-0
            # accumulators live in psA, image-1 in psB. ----
            ps1 = {}
            for i in range(2):
                for m in range(2):
                    pool = psA if i == 0 else psB
                    ps1[(m, i)] = pool.tile([128, 2, 512], f32, tag="ps",
                                            name=f"ps1_{m}{i}")
            for kt in range(8):
                for m, i in [(0, 0), (1, 0), (0, 1), (1, 1)]:
                    lhsT = w1_sb[:, kt * 2 + m]
                    for hb in range(2):
                        nc.tensor.matmul(
                            ps1[(m, i)][:, hb, 0:FB], lhsT,
                            x_sb[kt][:, i * HW_ + hb * FB:
                                      i * HW_ + (hb + 1) * FB],
                            start=(kt == 0), stop=(kt == 7))

            # stage-1/2 epilogue: A(m0) on DVE (u8 out, saturation = relu)
            # with ACT cast; A(m1) on ACT (relu -> i16) with DVE cast.
            def s12_epilogue(ps, i, a, b, dst, stage):
                r0 = sp.tile([128, 28, 28], u8, tag="r", name=f"r{stage}a_{i}")
                nc.vector.tensor_scalar(r0[:], ps[0][:, 0:2, 0:FB],
                                        a[0], b[0], Alu.mult, Alu.add)
                r1 = sp.tile([128, 28, 28], i16, tag="r", name=f"r{stage}b_{i}")
                nc.scalar.activation(r1[:], ps[1][:, 0:2, 0:FB],
                                     Act.Relu, bias=b[1], scale=a[1])
                if stage == 1:
                    d0, d1 = dst[:, 0, i, 1:29, 1:29], dst[:, 1, i, 1:29, 1:29]
                else:
                    d0 = dst[:, 0, 2 * i:2 * i + 2, 0:FB]
                    d1 = dst[:, 1, 2 * i:2 * i + 2, 0:FB]
                nc.scalar.activation(d0, r0[:], Act.Identity)
                nc.vector.tensor_copy(d1, r1[:])

            s12_epilogue([ps1[(0, 0)], ps1[(1, 0)]], 0, a1, b1, s1p, 1)
            s12_epilogue([ps1[(0, 1)], ps1[(1, 1)]], 1, a1, b1, s1p, 1)

            # ---- stage 2: fp8 DoubleRow 3x3 conv (K=256 -> M=256) ----
            def s2_taps(ps2, i, taps):
                for tap in taps:
                    dy, dx = tap // 3, tap % 3
                    for m in range(2):
                        lhsT = w2_sb[:, tap * 2 + m]
                        for hb in range(2):
                            h0 = hb * 14
                            rhs = s1p[:, :, i, h0 + dy:h0 + dy + 14, dx:dx + 28]
                            nc.tensor.matmul(
                                ps2[m][:, hb, 0:FB], lhsT, rhs,
                                start=(tap == 0), stop=(tap == 8), perf_mode=DR)

            # ---- stage 3 iteration: 2 DR matmuls + A (ACT psum->i16), and
            # separately-emitted B+C on DVE (r += x; min->u8).  Emission
            # order sets each engine's FIFO order. ----
            def s3_mm_a(i, m, pool):
                ps3 = pool.tile([128, 2, 512], f32, tag="ps", name=f"ps3_{m}{i}")
                for hb in range(2):
                    nc.tensor.matmul(ps3[:, hb, 0:FB], w3_sb[:, m],
                                     s2f[:, :, 2 * i + hb, 0:FB],
                                     start=True, stop=True, perf_mode=DR)
                r = sp3.tile([128, HW_], i16, tag="r3", name=f"r3_{m}{i}")
                nc.scalar.activation(r[:], ps3[:, 0:2, 0:FB],
                                     Act.Identity, bias=b3[m], scale=a3[m])
                return r

            def s3_bc(i, m, r):
                xs = x_sb[m][:, i * HW_:(i + 1) * HW_]
                nc.vector.tensor_tensor(r[:], r[:], xs, Alu.add)
                dst = out_sb[m][:, i * HW_:(i + 1) * HW_]
                nc.vector.tensor_scalar(dst, r[:], 127.0, None, Alu.min)
                if i == 1:
                    nc.sync.dma_start(out_d[m], out_sb[m][:])

            # stage-2 image 0 (all taps) + epilogue
            ps2_0 = [psA.tile([128, 2, 512], f32, tag="ps", name=f"ps2_{m}0")
                     for m in range(2)]
            s2_taps(ps2_0, 0, range(9))
            s12_epilogue(ps2_0, 0, a2, b2, s2f, 2)

            # stage-2 image 1 interleaved with stage-3 image 0
            ps2_1 = [psA.tile([128, 2, 512], f32, tag="ps", name=f"ps2_{m}1")
                     for m in range(2)]
            s2_taps(ps2_1, 1, range(0, 3))
            rs0 = [s3_mm_a(0, m, psB) for m in range(4)]
            for m in range(4):
                s3_bc(0, m, rs0[m])
            s2_taps(ps2_1, 1, range(3, 6))
            rs1 = [s3_mm_a(0, m, psB) for m in range(4, 8)]
            s2_taps(ps2_1, 1, range(6, 9))
            s12_epilogue(ps2_1, 1, a2, b2, s2f, 2)
            for m in range(4, 8):
                s3_bc(0, m, rs1[m - 4])

            # stage-3 image 1 (psum alternates psB / psA slots)
            for m in range(8):
                r = s3_mm_a(1, m, psB if m % 2 == 0 else psA)
                s3_bc(1, m, r)

    nc.compile()
    return nc


def _get_nc():
    if "nc" not in _CACHE:
        _CACHE["nc"] = _build()
    return _CACHE["nc"]


def _pack_inputs(inputs):
    """Host-side: effective weights, per-core shards, dtype casts."""
    f32 = np.float32
    x = np.asarray(inputs["x"])

    def eff(w2, s):
        return (np.asarray(w2, dtype=f32) *
                np.exp2(np.asarray(s).astype(f32))).astype(f32)

    # stage 1 (bf16): w1[p, kt*2+m, j] = W1_eff[kt*128+p, m*128+j]
    w1e = eff(inputs["w2_1"], inputs["s1"])[:, :, 0, 0]          # [O=256, I=1024]
    w1 = np.ascontiguousarray(
        w1e.T.reshape(8, 128, 2, 128).transpose(1, 0, 2, 3)     # [p, kt, m, j]
        .reshape(128, 16, 128)).astype(BF16)
    # stage 2 (fp8 pairs): logical W[b][p][j] = W2_eff[tap][b*128+p, m*128+j]
    w2e = eff(inputs["w2_2"], inputs["s2"])                      # [O, I, 3, 3]
    taps = np.stack([w2e[:, :, dy, dx].T                         # [I, O]
                     for dy in range(3) for dx in range(3)])     # [9, I=256, O=256]
    t5 = taps.reshape(9, 2, 128, 2, 128)                         # [tap, b, p, m, j]
    # stage 3 (fp8 pairs): logical W[b][p][j] = W3_eff[b*128+p, m*128+j]
    w3e = eff(inputs["w2_3"], inputs["s3"])[:, :, 0, 0]          # [O=1024, I=256]
    t3 = w3e.T.reshape(2, 128, 8, 128)                           # [b, p, m, j]
    # DRSW: phys[p, grp, 2c+b] = W[b][p][127-c] (A/B interleave, cols reversed)
    w2 = np.ascontiguousarray(
        t5[..., ::-1]                                            # j -> c=127-j
        .transpose(2, 0, 3, 4, 1)                                # [p, tap, m, c, b]
        .reshape(128, 18, 2, 128)).astype(FP8)
    w3 = np.ascontiguousarray(
        t3[..., ::-1].transpose(1, 2, 3, 0)                      # [p, m, c, b]
        .reshape(128, 8, 2, 128)).astype(FP8)

    vec = np.zeros((128, 24), dtype=f32)
    scl = np.exp2(f32(-12.0))
    for m in range(2):
        sl = slice(m * 128, (m + 1) * 128)
        vec[:, m] = np.asarray(inputs["alpha1"], dtype=f32)[sl] * scl
        vec[:, 2 + m] = (np.asarray(inputs["beta1"], dtype=f32)[sl] *
                         np.exp2(np.asarray(inputs["q1"]).astype(f32)[sl]))
        vec[:, 4 + m] = np.asarray(inputs["alpha2"], dtype=f32)[sl] * scl
        vec[:, 6 + m] = (np.asarray(inputs["beta2"], dtype=f32)[sl] *
                         np.exp2(np.asarray(inputs["q2"]).astype(f32)[sl]))
    for m in range(8):
        sl = slice(m * 128, (m + 1) * 128)
        vec[:, 8 + m] = np.asarray(inputs["alpha3"], dtype=f32)[sl] * scl
        vec[:, 16 + m] = (np.asarray(inputs["beta3"], dtype=f32)[sl] *
                          np.exp2(np.asarray(inputs["q3"]).astype(f32)[sl]))

    in_maps = []
    for c in range(N_CORES):
        xc = x[c * N_PER_CORE:(c + 1) * N_PER_CORE]              # [2, 1024, 28, 28]
        xc = np.ascontiguousarray(
            xc.transpose(1, 0, 2, 3).reshape(8, 128, F)).astype(BF16)
        in_maps.append({"x": xc, "w1": w1, "w2": w2, "w3": w3, "vec": vec})
    return in_maps


def _assemble(results):
    outs = []
    for c in range(N_CORES):
        o = results[c]["out"]                                    # [8,128,1568] u8
        o = o.reshape(1024, N_PER_CORE, 28, 28).transpose(1, 0, 2, 3)
        outs.append(o)
    return np.concatenate(outs, axis=0).astype(np.float32)


def _run(inputs, trace=False, **kwargs):
    from concourse.bass_utils import run_bass_kernel_spmd
    nc = _get_nc()
    in_maps = _pack_inputs(inputs)
    res = run_bass_kernel_spmd(nc, in_maps, list(range(N_CORES)),
                               trace=trace, **kwargs)
    return _assemble(res.results), res


def kernel(**inputs):
    out, _ = _run(inputs)
    return out
